# revision 1
# baseline (speedup 1.0000x reference)
"""Trainium2 Bass kernel for nn_BinaryMemoryRNN (scatter_memory).

Computation (reference):
    logits = h_prev @ Mw.T + Mb                 # [B, 28]
    b1/b2  = bits of logits halves (> 0)
    idx1   = clip(sum(b1 * 2^(13-j)), 0, 8191)
    idx2   = clip(sum(b2 * 2^(13-j)), 8192, 16383)
    pre    = x @ Ww.T + h_prev @ Uw.T + mem[idx1] @ Qrw.T + mem[idx2] @ Qlw.T + bias
    out    = sigmoid(layernorm(pre) * gamma + beta)

Strategy: data-parallel over batch across 8 cores (1024 rows each).
  - Activations pre-transposed on host to [feature, batch] layout (the PE
    contracts over the partition dim), bf16 for the 4 big matmuls.
  - logits matmul in fp32 (index bits are sign-sensitive).
  - memory table replicated in DRAM as bf16 [16384, 1024]; rows are fetched
    with gpsimd.dma_gather(transpose=True) which lands them directly in
    [feature, batch] layout.
  - LayerNorm + sigmoid epilogue on DVE/ACT per 128-row tile.
"""

import sys

sys.path.insert(0, "/opt/trn_rl_repo")

from contextlib import ExitStack

import numpy as np
import ml_dtypes

import concourse.bass as bass
import concourse.tile as tile
from concourse import bacc, mybir, library_config
from concourse.bass_utils import run_bass_kernel_spmd

F32 = mybir.dt.float32
BF16 = mybir.dt.bfloat16
I16 = mybir.dt.int16

B, I, H, NB = 8192, 1024, 1024, 14
MEM = 2**NB
NCORES = 8
BL = B // NCORES  # 1024 batch rows per core
KC = H // 128  # 8 contraction chunks
MT = BL // 128  # 8 output row-tiles per core
EPS = 1e-5

_CACHE = {}


def _build(trivial_gb: bool, dump_debug: bool = False, no_gather: bool = False):
    """Trace the Bass/Tile module (shared by all 8 cores, SPMD)."""
    nc = bacc.Bacc(
        "TRN2", target_bir_lowering=False, debug=False, enable_asserts=True
    )

    x_t = nc.dram_tensor("x_t", [128, KC, BL], BF16, kind="ExternalInput").ap()
    h_t32 = nc.dram_tensor("h_t32", [128, KC, BL], F32, kind="ExternalInput").ap()
    # weights, [src, feat_in(part), feat_in(chunk), feat_out]; src order W,U,Qr,Ql
    w_t = nc.dram_tensor("w_t", [4, 128, KC, H], BF16, kind="ExternalInput").ap()
    # packed consts: mw[0:224] | bias[224:1248] | pw-unused[1248:1250] |
    # clip[1250:1252] | negmb[1252:1253] | ident-as-f32[1253:1317] |
    # pw-as-bf16[1317:1318]
    NCONST = 1318
    const_t = nc.dram_tensor("const_t", [128, NCONST], F32, kind="ExternalInput").ap()
    mem_t = nc.dram_tensor("mem_t", [MEM, H], BF16, kind="ExternalInput").ap()
    if not trivial_gb:
        gam_t = nc.dram_tensor("gam_t", [128, H], F32, kind="ExternalInput").ap()
        bet_t = nc.dram_tensor("bet_t", [128, H], F32, kind="ExternalInput").ap()
    out_t = nc.dram_tensor("out_t", [BL, H], F32, kind="ExternalOutput").ap()
    if dump_debug:
        dbg_bits = nc.dram_tensor(
            "dbg_bits", [2 * NB, BL], F32, kind="ExternalOutput"
        ).ap()
        dbg_idx = nc.dram_tensor(
            "dbg_idx", [2, BL], I16, kind="ExternalOutput"
        ).ap()
        dbg_mem = nc.dram_tensor(
            "dbg_mem", [128, KC, BL], BF16, kind="ExternalOutput"
        ).ap()

    with tile.TileContext(nc) as tc:
        with ExitStack() as ctx:
            # ---------------- pools ----------------
            cpool = ctx.enter_context(tc.tile_pool(name="consts", bufs=1))
            apool = ctx.enter_context(tc.tile_pool(name="acts", bufs=1))
            # h32 halves and raw gathered tiles share 16KB/partition slots:
            # h32 dies after the logits matmul, before the gathers land.
            hpool = ctx.enter_context(tc.tile_pool(name="h32_or_gather", bufs=4))
            gpool = ctx.enter_context(tc.tile_pool(name="gathered", bufs=1))
            spool = ctx.enter_context(tc.tile_pool(name="small", bufs=2))
            epool = ctx.enter_context(tc.tile_pool(name="epilogue", bufs=2))
            pp_main = ctx.enter_context(
                tc.tile_pool(name="psum_main", bufs=2, space="PSUM")
            )
            # logits / idx / PE-transpose outputs share two 2-bank slots
            pp_small = ctx.enter_context(
                tc.tile_pool(name="psum_small", bufs=2, space="PSUM")
            )

            # gpsimd ucode library containing DMAGatherAnt; load it up front
            # so the Q7 IRAM reload overlaps the initial DMAs.
            nc.gpsimd.load_library(library_config.attnmlp)

            # ---------------- input loads ----------------
            # critical path first: packed consts + h fp32 for the index pipeline
            const_sb = cpool.tile([128, NCONST], F32, tag="const")
            nc.sync.dma_start(const_sb[:], const_t[:])
            mw_sb = const_sb[:, 0:224].rearrange("p (k j) -> p k j", j=2 * NB)
            bias_sb = const_sb[:, 224:1248]
            pw_sb = const_sb[0 : 2 * NB, 1317:1318].bitcast(BF16)
            clip_sb = const_sb[0:2, 1250:1252]
            negmb_sb = const_sb[0 : 2 * NB, 1252:1253]
            ident_sb = const_sb[:, 1253:1317].bitcast(BF16)
            eps_sb = cpool.tile([128, 1], F32, tag="eps")
            nc.vector.memset(eps_sb[:], EPS)

            # h32 split 1/2/2/2/1 chunks: the logits matmul starts after the
            # first 512KB, and the 8KB middle pieces share pool slots with
            # the half-gather destinations later
            h32_k0 = spool.tile([128, 1, BL], F32, tag="h32k0")
            nc.sync.dma_start(h32_k0[:], h_t32[:, 0:1, :])
            h32_mid = []
            for piece in range(3):
                hp = hpool.tile([128, 2, BL], F32, tag="slab")
                nc.sync.dma_start(
                    hp[:], h_t32[:, 1 + 2 * piece : 3 + 2 * piece, :]
                )
                h32_mid.append(hp)
            h32_k7 = spool.tile([128, 1, BL], F32, tag="h32k7")
            nc.sync.dma_start(h32_k7[:], h_t32[:, KC - 1 : KC, :])

            def h32_chunk(k):
                if k == 0:
                    return h32_k0[:, 0, :]
                if k == KC - 1:
                    return h32_k7[:, 0, :]
                return h32_mid[(k - 1) // 2][:, (k - 1) % 2, :]

            x_sb = apool.tile([128, KC, BL], BF16, tag="x")
            nc.sync.dma_start(x_sb[:], x_t[:])
            # h16 derived on-device from h32 (gpsimd is idle; saves a 2MB load)
            h16_sb = apool.tile([128, KC, BL], BF16, tag="h16")
            nc.gpsimd.tensor_copy(h16_sb[:, 0:1, :], h32_k0[:])
            for piece in range(3):
                nc.gpsimd.tensor_copy(
                    h16_sb[:, 1 + 2 * piece : 3 + 2 * piece, :], h32_mid[piece][:]
                )
            nc.gpsimd.tensor_copy(h16_sb[:, KC - 1 : KC, :], h32_k7[:])
            # W and U weights now; Qr/Ql weights are loaded later so they
            # don't queue ahead of latency-critical small DMAs
            w_sb = []
            for s in range(4):
                w = cpool.tile([128, KC, H], BF16, tag=f"w{s}")
                if s < 2:
                    nc.sync.dma_start(w[:], w_t[s])
                w_sb.append(w)

            def act_slice(tiles, k, sl):
                return tiles[:, k, sl]
            if not trivial_gb:
                gam_sb = cpool.tile([128, H], F32, tag="gam")
                nc.sync.dma_start(gam_sb[:], gam_t[:])
                bet_sb = cpool.tile([128, H], F32, tag="bet")
                nc.sync.dma_start(bet_sb[:], bet_t[:])

            # ---------------- index pipeline ----------------
            # logits.T [28, BL] fp32, accumulated over KC chunks; k-outer so
            # the first half of h32 is enough to start
            logit_ps = pp_small.tile([2 * NB, BL], F32, tag="sm")
            for k in range(KC):
                hk = h32_chunk(k)
                for n in range(BL // 512):
                    nc.tensor.matmul(
                        logit_ps[:, n * 512 : (n + 1) * 512],
                        mw_sb[:, k, :],
                        hk[:, n * 512 : (n + 1) * 512],
                        start=(k == 0),
                        stop=(k == KC - 1),
                    )
            # bits = (h@Mw.T + Mb > 0)  <=>  (h@Mw.T > -Mb), as 1.0/0.0
            # (bf16: exact for 0/1, and the powers matmul sums stay exact
            # in fp32 PSUM)
            bits_sb = spool.tile([2 * NB, BL], BF16, tag="bits")
            nc.vector.tensor_scalar(
                bits_sb[:], logit_ps[:], negmb_sb[:, 0:1], None,
                mybir.AluOpType.is_gt,
            )
            # raw indices via tiny matmul with powers of two: [2, BL]
            idx_ps = pp_small.tile([2, BL], F32, tag="sm")
            for n in range(BL // 512):
                nc.tensor.matmul(
                    idx_ps[:, n * 512 : (n + 1) * 512],
                    pw_sb,
                    bits_sb[:, n * 512 : (n + 1) * 512],
                    start=True,
                    stop=True,
                )
            # clip + cast to int16 (values are exact integers in fp32);
            # per-partition clip bounds: row0 -> [0, 8191], row1 -> [8192, 16383]
            idx16 = spool.tile([2, BL], I16, tag="idx16")
            nc.vector.tensor_scalar(
                idx16[:], idx_ps[:], clip_sb[:, 0:1], clip_sb[:, 1:2],
                mybir.AluOpType.max, mybir.AluOpType.min,
            )

            # Wrap each index row into the [16, BL/16] layout dma_gather wants,
            # replicated to every 16-partition group (the Q7 ucode cores each
            # read their own group). Stage S[i, 32j+q'] = idx[(32j+i)*16+q'%16]
            # (16 columns duplicated within each 32-block), then four DVE
            # 32x32 block-transposes to partition bases 0/32/64/96.
            idxw_r = []
            for r in range(2):
                # issue on ACT's HWDGE FIFO so this tiny latency-critical
                # transfer doesn't queue behind the big input loads on SP's;
                # strided DMAs fill cols {0:16, 32:48}, a DVE copy
                # duplicates into cols {16:32, 48:64}
                stg = spool.tile([32, 64], I16, tag="stage")
                stg_j = stg[0:32, :].rearrange("p (j hq) -> p j hq", j=2)
                with nc.allow_non_contiguous_dma(reason="tiny idx wrap staging"):
                    for j in range(2):
                        nc.scalar.dma_start(
                            stg[0:32, 32 * j : 32 * j + 16],
                            idx16[r : r + 1, j * 512 : (j + 1) * 512].rearrange(
                                "p (a b) -> p a b", b=16
                            ),
                        )
                nc.vector.tensor_copy(stg_j[:, :, 16:32], stg_j[:, :, 0:16])
                idxw = spool.tile([128, 64], I16, tag="idxw")
                for g in range(4):
                    nc.vector.transpose(idxw[32 * g : 32 * (g + 1), :], stg[:])
                idxw_r.append(idxw)

            # gathers split in batch halves, interleaved r0/r1, so blocks
            # c=0-3 of BOTH tensors arrive after the first two half-gathers.
            # g2[r][hf][p, c, :] = mem[idx_{(4*hf+c)*128+p}, :]
            HB = BL // 2
            g2_tiles = [[None, None], [None, None]]
            for hf in range(2):
                for r in range(2):
                    g2 = hpool.tile([128, HB // 128, H], BF16, tag="slab")
                    if no_gather:
                        nc.sync.dma_start(
                            g2[:],
                            mem_t.rearrange("(a p) h -> p a h", p=128)[
                                :, 0 : HB // 128, :
                            ],
                        )
                    else:
                        nc.gpsimd.dma_gather(
                            out_ap=g2[:],
                            in_ap=mem_t[:],
                            idxs_ap=idxw_r[r][:, hf * 32 : (hf + 1) * 32],
                            num_idxs=HB,
                            num_idxs_reg=HB,
                            elem_size=H,
                            transpose=False,
                        )
                    g2_tiles[r][hf] = g2

            # Qr/Ql weights: needed only once the mem matmuls start
            for s in (2, 3):
                nc.sync.dma_start(w_sb[s][:], w_t[s])

            # ---------------- main matmuls + epilogue ----------------
            # Emission order = PE stream order: x/h matmuls for the first two
            # row-tiles run while the gather is in flight; then PE-transposes
            # of the gathered rows; then the mem matmuls + epilogues pipeline
            # with the remaining x/h matmuls.
            srcs_xh = [(x_sb, 0), (h16_sb, 1)]
            ps_tiles = {}

            def emit_xh(m):
                ps = pp_main.tile([128, H], F32, tag="acc")
                ps_tiles[m] = ps
                ms = slice(m * 128, (m + 1) * 128)
                for si, (act, wi) in enumerate(srcs_xh):
                    for k in range(KC):
                        lhs = act_slice(act, k, ms)
                        for n in range(H // 512):
                            nc.tensor.matmul(
                                ps[:, n * 512 : (n + 1) * 512],
                                lhs,
                                act_slice(
                                    w_sb[wi], k, slice(n * 512, (n + 1) * 512)
                                ),
                                start=(si == 0 and k == 0),
                                stop=False,
                            )

            def emit_mem_epilogue(m):
                ps = ps_tiles.pop(m)
                ms = slice(m * 128, (m + 1) * 128)
                for si in range(2):
                    mt = mem_sb[si][m]  # [128, KC, 128] block for this m
                    for k in range(KC):
                        lhs = mt[:, k, :]
                        for n in range(H // 512):
                            nc.tensor.matmul(
                                ps[:, n * 512 : (n + 1) * 512],
                                lhs,
                                act_slice(
                                    w_sb[2 + si], k, slice(n * 512, (n + 1) * 512)
                                ),
                                start=False,
                                stop=(si == 1 and k == KC - 1),
                            )

                # t = pre + bias  (bias varies along the free/feature dim)
                t = epool.tile([128, H], F32, tag="t")
                nc.vector.tensor_tensor(
                    t[:], ps[:], bias_sb[:], mybir.AluOpType.add
                )
                # layernorm stats
                st6 = epool.tile([128, 2, 6], F32, tag="st6")
                for a in range(2):
                    nc.vector.bn_stats(st6[:, a, :], t[:, a * 512 : (a + 1) * 512])
                mv = epool.tile([128, 2], F32, tag="mv")
                nc.vector.bn_aggr(mv[:], st6.rearrange("p a b -> p (a b)"))
                # rstd = 1/sqrt(var + eps): ACT sqrt, then the fast custom-DVE
                # reciprocal (~18 correct bits, plenty for layernorm).
                # sc[:,0] holds std then -mu*rstd; sc[:,1] holds rstd.
                sc = epool.tile([128, 2], F32, tag="sc")
                nc.scalar.activation(
                    sc[:, 0:1], mv[:, 1:2], mybir.ActivationFunctionType.Sqrt,
                    bias=eps_sb[:, 0:1],
                )
                nc.vector.reciprocal_approx_fast(sc[:, 1:2], sc[:, 0:1])
                nc.vector.tensor_scalar(
                    sc[:, 0:1], mv[:, 0:1], sc[:, 1:2], -1.0,
                    mybir.AluOpType.mult, mybir.AluOpType.mult,
                )
                rstd = sc[:, 1:2]
                nmu = sc[:, 0:1]
                o = epool.tile([128, H], F32, tag="o")
                if trivial_gb:
                    # out = sigmoid((t - mu) * rstd)
                    nc.scalar.activation(
                        o[:], t[:], mybir.ActivationFunctionType.Sigmoid,
                        bias=nmu[:, 0:1], scale=rstd[:, 0:1],
                    )
                else:
                    xh = epool.tile([128, H], F32, tag="xh")
                    nc.scalar.activation(
                        xh[:], t[:], mybir.ActivationFunctionType.Identity,
                        bias=nmu[:, 0:1], scale=rstd[:, 0:1],
                    )
                    nc.vector.tensor_tensor(
                        xh[:], xh[:], gam_sb[:], mybir.AluOpType.mult
                    )
                    nc.vector.tensor_tensor(
                        xh[:], xh[:], bet_sb[:], mybir.AluOpType.add
                    )
                    zero_sb = cpool.tile([128, 1], F32, tag="zero")
                    nc.vector.memset(zero_sb[:], 0.0)
                    nc.scalar.activation(
                        o[:], xh[:], mybir.ActivationFunctionType.Sigmoid,
                        bias=zero_sb[:, 0:1],
                    )
                nc.sync.dma_start(out_t[ms, :], o[:])

            emit_xh(0)
            emit_xh(1)

            # PE-transpose gathered rows into [feat, batch] layout; one tile
            # per (tensor, batch-block) so each m-tile's mem matmuls depend
            # only on its own block's transposes
            mem_sb = [[], []]
            for c in range(BL // 128):
                for r in range(2):
                    g2 = g2_tiles[r][c // 4]
                    cc = c % 4
                    mt = gpool.tile([128, KC, 128], BF16, tag=f"mem{r}_{c}")
                    for k in range(KC):
                        tp = pp_small.tile([128, 128], BF16, tag="sm")
                        nc.tensor.transpose(
                            tp[:], g2[:, cc, k * 128 : (k + 1) * 128], ident_sb[:]
                        )
                        nc.vector.tensor_copy(mt[:, k, :], tp[:])
                    mem_sb[r].append(mt)

            if dump_debug:
                nc.sync.dma_start(dbg_bits[:], bits_sb[:])
                nc.sync.dma_start(dbg_idx[:], idx16[:])
                for c in range(BL // 128):
                    nc.sync.dma_start(
                        dbg_mem[:, :, c * 128 : (c + 1) * 128], mem_sb[0][c][:]
                    )

            emit_mem_epilogue(0)
            for m in range(2, MT):
                emit_xh(m)
                emit_mem_epilogue(m - 1)
            emit_mem_epilogue(MT - 1)

    nc.compile()  # bacc register allocation / DCE
    return nc


def _to_kxp(a, dtype):
    """[batch, feat] -> [128, KC, batch] with feat = k*128 + p."""
    t = np.ascontiguousarray(a.T.reshape(KC, 128, -1).transpose(1, 0, 2))
    return t.astype(dtype)


def prep(inputs):
    """Host-side shard/layout prep. Returns (in_maps, trivial_gb)."""
    x = np.asarray(inputs["x"], np.float32)
    h = np.asarray(inputs["h_prev"], np.float32)
    memory = np.asarray(inputs["memory"], np.float32)
    gamma = np.asarray(inputs["gamma"], np.float32)
    beta = np.asarray(inputs["beta"], np.float32)
    trivial_gb = bool(np.all(gamma == 1.0) and np.all(beta == 0.0))

    bf = ml_dtypes.bfloat16
    # W is [out, in]; the kernel wants w[p, k, n] = W[n, k*128+p], which is
    # exactly _to_kxp applied to W with (out, in) in the (batch, feat) slots.
    w_cat = np.stack(
        [_to_kxp(np.asarray(inputs[n], np.float32), bf) for n in ("Ww", "Uw", "Qrw", "Qlw")]
    )
    mw = _to_kxp(np.asarray(inputs["Mw"], np.float32), np.float32)  # [128, KC, 28]

    pw = np.zeros((2 * NB, 2), np.float32)
    pw[:NB, 0] = 2.0 ** np.arange(NB - 1, -1, -1)
    pw[NB:, 1] = 2.0 ** np.arange(NB - 1, -1, -1)
    clip = np.array(
        [[0.0, MEM // 2 - 1], [MEM // 2, MEM - 1]], np.float32
    )  # [row, (lo, hi)]

    mem_bf = memory.astype(bf)
    ident = np.eye(128, dtype=np.float32).astype(bf)
    bias = (
        np.asarray(inputs["Wb"], np.float32)
        + np.asarray(inputs["Ub"], np.float32)
        + np.asarray(inputs["Qrb"], np.float32)
        + np.asarray(inputs["Qlb"], np.float32)
    )

    # pack all small constants into one [128, 1318] f32 buffer (single DMA)
    const = np.zeros((128, 1318), np.float32)
    const[:, 0:224] = mw.reshape(128, 224)
    const[:, 224:1248] = np.broadcast_to(bias, (128, H))
    const[: 2 * NB, 1248:1250] = pw
    const[:2, 1250:1252] = clip
    const[: 2 * NB, 1252:1253] = -np.asarray(inputs["Mb"], np.float32).reshape(
        2 * NB, 1
    )
    const[:, 1253:1317] = ident.view(np.float32)
    const[: 2 * NB, 1317:1318] = pw.astype(bf).view(np.float32)

    common = dict(w_t=w_cat, const_t=const, mem_t=mem_bf)
    if not trivial_gb:
        common["gam_t"] = np.ascontiguousarray(np.broadcast_to(gamma, (128, H)))
        common["bet_t"] = np.ascontiguousarray(np.broadcast_to(beta, (128, H)))

    in_maps = []
    for c in range(NCORES):
        xs = x[c * BL : (c + 1) * BL]
        hs = h[c * BL : (c + 1) * BL]
        in_maps.append(
            dict(x_t=_to_kxp(xs, bf), h_t32=_to_kxp(hs, np.float32), **common)
        )
    return in_maps, trivial_gb


def get_nc(trivial_gb):
    key = ("nc", trivial_gb)
    if key not in _CACHE:
        _CACHE[key] = _build(trivial_gb)
    return _CACHE[key]


def run(inputs, trace=False, **kw):
    in_maps, trivial_gb = prep(inputs)
    nc = get_nc(trivial_gb)
    res = run_bass_kernel_spmd(
        nc, in_maps, core_ids=list(range(NCORES)), trace=trace, **kw
    )
    out = np.concatenate([res.results[c]["out_t"] for c in range(NCORES)], axis=0)
    return out.astype(np.float32), res


def kernel(**inputs):
    return run(inputs)[0]

